# revision 36
# baseline (speedup 1.0000x reference)
"""Trainium2 Bass kernel for nn_AttentionHead (B=8, S=2048, DK=512).

Reference semantics (faithful to the source module, bugs included):
    qh = q @ Wq.T + bq            # [B, S, D]
    kh = k @ Wk.T + bk
    vh = v @ Wv.T + bv
    kr = kh.reshape(B, D, S)      # row-major REINTERPRET, not a transpose
    s  = (qh @ kr) * sqrt(D)      # source bug: multiplies by sqrt(D)
    a  = softmax(s, axis=2)
    out = a @ vh                  # [B, S, D]

Sharding: data-parallel over batch — one batch element per NeuronCore,
8 cores, no collectives. Each core runs the same NEFF with its own shard.

Per-core dataflow:
  - q,k,v are PE-transposed (fp32 transpose-mode matmuls) to put the
    contraction dim d on partitions.
  - qh is produced transposed ([e_part, i]) so it can feed the score
    matmul as the stationary operand directly.
  - kh is produced in natural [s, e] layout, spilled to DRAM, and
    re-loaded as kr = reshape view (fully contiguous DMA both ways).
  - scores: s[i_part, j] accumulated in PSUM with fp32r (tf32) matmuls.
  - softmax: per-512-chunk max + exp (ScalarE, accum_out gives sums);
    chunk-vs-global-max correction factor applied to the bf16 prob tiles;
    1/denominator folded into the output copy.
  - probs are PE-transposed (bf16) and a@vh runs in bf16 with fp32
    accumulation; vh is kept in bf16.
"""

from contextlib import ExitStack

import numpy as np

import concourse.bacc as bacc
import concourse.mybir as mybir
import concourse.tile as tile
from concourse.bass_utils import run_bass_kernel_spmd
from concourse.masks import make_identity

AF = mybir.ActivationFunctionType
ALU = mybir.AluOpType
AX = mybir.AxisListType
F32 = mybir.dt.float32
F32R = mybir.dt.float32r
BF16 = mybir.dt.bfloat16

B, S, D = 8, 2048, 512
P = 128
NT_S = S // P          # 16 s-tiles (also j-tiles / i-blocks)
NT_D = D // P          # 4 d-tiles (also e-tiles)
NCH = S // 512         # 4 512-wide chunks of the sequence dim
SQRT_D = float(np.sqrt(np.float32(D)))


def build_nc():
    nc = bacc.Bacc("TRN2", target_bir_lowering=False, debug=False,
                   enable_asserts=False, num_devices=B)

    q = nc.dram_tensor("q", [S, D], F32, kind="ExternalInput").ap()
    k = nc.dram_tensor("k", [S, D], F32, kind="ExternalInput").ap()
    v = nc.dram_tensor("v", [S, D], F32, kind="ExternalInput").ap()
    Wq = nc.dram_tensor("Wq", [D, D], F32, kind="ExternalInput").ap()
    Wk = nc.dram_tensor("Wk", [D, D], F32, kind="ExternalInput").ap()
    Wv = nc.dram_tensor("Wv", [D, D], F32, kind="ExternalInput").ap()
    bq = nc.dram_tensor("bq", [D], F32, kind="ExternalInput").ap()
    bk = nc.dram_tensor("bk", [D], F32, kind="ExternalInput").ap()
    bv = nc.dram_tensor("bv", [D], F32, kind="ExternalInput").ap()
    out = nc.dram_tensor("out", [S, D], F32, kind="ExternalOutput").ap()

    with tile.TileContext(nc) as tc:
        _build(nc, tc, q, k, v, Wq, Wk, Wv, bq, bk, bv, out)
    nc.compile()
    return nc


def _build(nc, tc, q, k, v, Wq, Wk, Wv, bq, bk, bv, out):
    with ExitStack() as ctx:
        _build_inner(nc, tc, ctx, q, k, v, Wq, Wk, Wv, bq, bk, bv, out)


def _build_inner(nc, tc, ctx, q, k, v, Wq, Wk, Wv, bq, bk, bv, out):
    # ---- pools -------------------------------------------------------
    const = ctx.enter_context(tc.tile_pool(name="const", bufs=1))
    resid = ctx.enter_context(tc.tile_pool(name="resid", bufs=1))
    io = ctx.enter_context(tc.tile_pool(name="io", bufs=6))
    work = ctx.enter_context(tc.tile_pool(name="work", bufs=3))
    stats = ctx.enter_context(tc.tile_pool(name="stats", bufs=3))
    psT = ctx.enter_context(tc.tile_pool(name="psT", bufs=3, space="PSUM"))
    psS = ctx.enter_context(tc.tile_pool(name="psS", bufs=3, space="PSUM"))
    psO = ctx.enter_context(tc.tile_pool(name="psO", bufs=2, space="PSUM"))

    # ---- prefetch first k-tile group (heads the DMA ring) ------------
    k_t = k.rearrange("(t c p) d -> p t c d", p=P, c=4)
    k_pre = []
    for c in range(4):
        xn_p = io.tile([P, D], F32R, tag="xn", name="kn_pre")
        nc.sync.dma_start(xn_p[:], k_t[:, 0, c, :].bitcast(F32R))
        k_pre.append(xn_p)

    # ---- PE warm-up: dummy matmuls fill the initial DMA wait and bring
    # the HAM clock to 2.4 GHz before real work arrives (values unused).
    warm = const.tile([P, D], BF16, name="warm")
    nc.gpsimd.memset(warm[:], 0.0)
    wps = psO.tile([P, D], F32, tag="o512", name="warm_ps")
    for _ in range(20):
        nc.tensor.matmul(wps[:], warm[:, :P], warm[:], start=True, stop=True)

    # ---- constants ---------------------------------------------------
    ident_f32 = const.tile([P, P], F32, name="ident_f32")
    make_identity(nc, ident_f32)
    ident_f32r_t = const.tile([P, P], F32R, name="ident_f32r_t")
    nc.vector.tensor_copy(ident_f32r_t[:], ident_f32[:])
    ident_f32r = ident_f32r_t[:]
    ident_bf16 = const.tile([P, P], BF16, name="ident_bf16")
    make_identity(nc, ident_bf16)



    # ---- weights: load natural [e_part, d] and PE-transpose to [d_part, e]
    WT = {}

    def load_wt(name, W):
        Wn = work.tile([P, NT_D, D], F32R, tag="wn", name=f"Wn_{name}", bufs=1)
        W_r = W.rearrange("(t p) d -> p t d", p=P).bitcast(F32R)
        for et in range(NT_D):
            nc.sync.dma_start(Wn[:, et, :], W_r[:, et, :])
        WTt = resid.tile([P, NT_D, D], F32R, tag=f"WT_{name}", name=f"WT_{name}")
        for et in range(NT_D):
            ps = psT.tile([P, NT_D, P], F32R, tag="ps128", name="wt_ps")
            for dt in range(NT_D):
                nc.tensor.transpose(ps[:, dt, :],
                                    Wn[:, et, dt * P:(dt + 1) * P],
                                    ident_f32r)
            nc.scalar.copy(WTt[:, :, et * P:(et + 1) * P], ps[:])
        WT[name] = WTt

    load_wt("k", Wk)

    # biases after Wk on the ring (not needed until the first psum lands)
    bqT = const.tile([P, NT_D], F32, name="bqT")
    nc.sync.dma_start(bqT[:], bq.rearrange("(t p) -> p t", p=P))
    bkb = const.tile([P, D], F32, name="bkb")
    nc.sync.dma_start(bkb[:], bk[None, :].to_broadcast((P, D)))
    bvb = const.tile([P, D], F32, name="bvb")
    nc.sync.dma_start(bvb[:], bv[None, :].to_broadcast((P, D)))

    # ---- residents ---------------------------------------------------
    qhT = resid.tile([P, NT_D, S], F32R, tag="qhT", name="qhT")      # [e, i]
    vh = resid.tile([P, NT_S, D], BF16, tag="vh", name="vh")        # [j, e]
    kr = resid.tile([P, NT_D, S], F32R, tag="kr", name="kr")         # [e', j]

    # ---- kh projection straight into kr ------------------------------
    # For kr row-block t2 (e' in [128*t2, 128*t2+128)) and lane a in [0,4):
    # psum partition p must hold kh row s = 512*t2 + 4*p + a, which is
    # column (4*p + a) of the transposed 4-tile group -> stride-4 lhsT AP.
    for t2 in range(NT_D):
        xTg = work.tile([P, NT_D, 512], F32R, tag="qTc", name="kTg", bufs=2)
        for c in range(4):
            if t2 == 0:
                xn = k_pre[c]
            else:
                xn = io.tile([P, D], F32R, tag="xn", name="kn")
                nc.sync.dma_start(xn[:], k_t[:, t2, c, :].bitcast(F32R))
            ps = psT.tile([P, NT_D, P], F32R, tag="ps128", name="kt_ps")
            for dt in range(NT_D):
                nc.tensor.transpose(ps[:, dt, :],
                                    xn[:, dt * P:(dt + 1) * P],
                                    ident_f32r)
            nc.scalar.copy(xTg[:, :, c * P:(c + 1) * P], ps[:])
        for a in range(4):
            pp = psS.tile([P, D], F32, tag="ps512", name="kh_ps")
            for dt in range(NT_D):
                nc.tensor.matmul(pp[:], xTg[:, dt, a::4],
                                 WT["k"][:, dt, :],
                                 start=(dt == 0), stop=(dt == NT_D - 1))
            nc.vector.tensor_tensor(kr[:, t2, a * 512:(a + 1) * 512], pp[:],
                                    bkb[:], op=ALU.add)

    # ---- qh^T projection ([e_part, i]); q transpose fused ------------
    q_t = q.rearrange("(c s p) d -> p c s d", p=P, c=NCH)  # c: i-chunk, s: subtile

    def qh_chunk(ic):
        qTc = work.tile([P, NT_D, 512], F32R, tag="qTc", name="qTc", bufs=2)
        for c in range(4):
            xn = io.tile([P, D], F32R, tag="xn", name="qn")
            nc.sync.dma_start(xn[:], q_t[:, ic, c, :].bitcast(F32R))
            ps = psT.tile([P, NT_D, P], F32R, tag="ps128", name="qt_ps")
            for dt in range(NT_D):
                nc.tensor.transpose(ps[:, dt, :],
                                    xn[:, dt * P:(dt + 1) * P],
                                    ident_f32r)
            nc.scalar.copy(qTc[:, :, c * P:(c + 1) * P], ps[:])
        for et in range(NT_D):
            pp = psS.tile([P, 512], F32, tag="ps512", name="qh_ps")
            for dt in range(NT_D):
                nc.tensor.matmul(pp[:], WT["q"][:, dt, et * P:(et + 1) * P],
                                 qTc[:, dt, :],
                                 start=(dt == 0), stop=(dt == NT_D - 1))
            nc.scalar.activation(qhT[:, et, ic * 512:(ic + 1) * 512], pp[:],
                                 AF.Identity, bias=bqT[:, et:et + 1], scale=1.0)

    # ---- vh projection (natural layout, bf16) ------------------------
    load_wt("v", Wv)
    v_t = v.rearrange("(t p) d -> p t d", p=P)
    for st in range(NT_S):
        xn = io.tile([P, D], F32R, tag="xn", name="vn")
        nc.sync.dma_start(xn[:], v_t[:, st, :].bitcast(F32R))
        xT = work.tile([P, NT_D, P], F32R, tag="xT", name="xT")
        ps = psT.tile([P, NT_D, P], F32R, tag="ps128", name="vt_ps")
        for dt in range(NT_D):
            nc.tensor.transpose(ps[:, dt, :],
                                xn[:, dt * P:(dt + 1) * P],
                                ident_f32r)
        nc.scalar.copy(xT[:], ps[:])
        pp = psS.tile([P, D], F32, tag="ps512", name="vh_ps")
        for dt in range(NT_D):
            nc.tensor.matmul(pp[:], xT[:, dt, :],
                             WT["v"][:, dt, :],
                             start=(dt == 0), stop=(dt == NT_D - 1))
        nc.vector.tensor_tensor(vh[:, st, :], pp[:], bvb[:], op=ALU.add)

    load_wt("q", Wq)
    for ic in range(NCH):
        qh_chunk(ic)

    # ---- attention per 128-row i-block -------------------------------
    for ib in range(NT_S):
        mx = stats.tile([P, NCH], F32, tag="mx", name="mx")
        ssum = stats.tile([P, NCH], F32, tag="ssum", name="ssum")
        p_bf = work.tile([P, S], BF16, tag="p", name="p_bf", bufs=3)

        s_sb = work.tile([P, NCH, 512], F32, tag="s_sb", name="s_sb", bufs=3)
        for jc in range(NCH):
            sp = psS.tile([P, 512], F32, tag="ps512", name="s_ps")
            for et in range(NT_D):
                nc.tensor.matmul(sp[:],
                                 qhT[:, et, ib * P:(ib + 1) * P],
                                 kr[:, et, jc * 512:(jc + 1) * 512],
                                 start=(et == 0), stop=(et == NT_D - 1))
            nc.vector.tensor_copy(s_sb[:, jc, :], sp[:])
            nc.vector.reduce_max(mx[:, jc:jc + 1], sp[:], axis=AX.X)

        gmx = stats.tile([P, 1], F32, tag="gmx", name="gmx")
        ngmx = stats.tile([P, 1], F32, tag="ngmx", name="ngmx")
        den = stats.tile([P, 1], F32, tag="den", name="den")
        rs = stats.tile([P, 1], F32, tag="rs", name="rs")
        nc.vector.reduce_max(gmx[:], mx[:], axis=AX.X)
        nc.vector.tensor_scalar_mul(ngmx[:], gmx[:], -SQRT_D)

        for jc in range(NCH):
            nc.scalar.activation(p_bf[:, jc * 512:(jc + 1) * 512],
                                 s_sb[:, jc, :], AF.Exp, bias=ngmx[:, 0:1],
                                 scale=SQRT_D, accum_out=ssum[:, jc:jc + 1])
        nc.vector.reduce_sum(den[:], ssum[:], axis=AX.X)
        nc.vector.reciprocal(rs[:], den[:])

        pT = work.tile([P, NT_S, P], BF16, tag="pT", name="pT", bufs=2)
        for a in range(NCH):
            ps = psT.tile([P, NT_D, P], BF16, tag="ps128", name="pt_ps")
            for b2 in range(NT_D):
                jt = a * NT_D + b2
                nc.tensor.transpose(ps[:, b2, :], p_bf[:, jt * P:(jt + 1) * P],
                                    ident_bf16[:])
            nc.vector.tensor_copy(pT[:, a * NT_D:(a + 1) * NT_D, :], ps[:])

        op = psO.tile([P, D], F32, tag="o512", name="o_ps")
        for jt in range(NT_S):
            nc.tensor.matmul(op[:], pT[:, jt, :], vh[:, jt, :],
                             start=(jt == 0), stop=(jt == NT_S - 1))
        o_sb = work.tile([P, D], F32, tag="o_sb", name="o_sb")
        nc.scalar.activation(o_sb[:], op[:], AF.Copy, bias=0.0,
                             scale=rs[:, 0:1])
        nc.sync.dma_start(out.rearrange("(t p) e -> p t e", p=P)[:, ib, :],
                          o_sb[:])


def _ensure_axon_hooks_module():
    """antenv.axon_hooks is missing on this image; provide it (with the real
    ctypes NTFF hook when available) so run_bass_kernel_spmd(trace=True)
    degrades gracefully instead of raising ImportError."""
    import sys
    import types
    try:
        import antenv
        import antenv.axon_hooks  # noqa: F401
        return
    except ImportError:
        pass
    try:
        mod = types.ModuleType("antenv.axon_hooks")
        state = {"hook": None}
        mod.set_axon_ntff_profile_hook = lambda h: state.__setitem__("hook", h)
        mod.get_axon_ntff_profile_hook = lambda: state["hook"]
        sys.modules["antenv.axon_hooks"] = mod
        antenv.axon_hooks = mod
        try:
            if "/root/.axon_site" not in sys.path:
                sys.path.insert(0, "/root/.axon_site")
            from trn_agent_boot.trn_boot import _ntff_profile_via_ctypes

            mod.set_axon_ntff_profile_hook(
                _ntff_profile_via_ctypes("/opt/axon/libaxon_pjrt.so")
            )
        except Exception:
            pass
    except Exception:
        pass


_ensure_axon_hooks_module()

_NC_CACHE = None


def _get_nc():
    global _NC_CACHE
    if _NC_CACHE is None:
        _NC_CACHE = build_nc()
    return _NC_CACHE


def kernel(q, k, v, Wq, bq, Wk, bk, Wv, bv):
    nc = _get_nc()
    in_maps = []
    for b in range(B):
        in_maps.append({
            "q": np.ascontiguousarray(q[b], dtype=np.float32),
            "k": np.ascontiguousarray(k[b], dtype=np.float32),
            "v": np.ascontiguousarray(v[b], dtype=np.float32),
            "Wq": np.ascontiguousarray(Wq, dtype=np.float32),
            "Wk": np.ascontiguousarray(Wk, dtype=np.float32),
            "Wv": np.ascontiguousarray(Wv, dtype=np.float32),
            "bq": np.ascontiguousarray(bq, dtype=np.float32),
            "bk": np.ascontiguousarray(bk, dtype=np.float32),
            "bv": np.ascontiguousarray(bv, dtype=np.float32),
        })
    res = run_bass_kernel_spmd(nc, in_maps, core_ids=list(range(B)))
    return np.stack([res.results[b]["out"] for b in range(B)], axis=0)
